# revision 54
# baseline (speedup 1.0000x reference)
"""TRN2 Bass kernel for nn_AttentionBlock (B=2,S=2048,E=2048,H=32,KV=8,D=64).

Sharding: 8 cores = 4 head-groups x 2 batch. Each core computes 8 q-heads /
2 kv-heads of one batch element, producing a partial (S,E) output; host sums
the 4 group partials per batch.

Per-core dataflow (all matmuls in fp32r: 1 cycle/row at N>=512):
  xT (E,S) -> [proj] -> QT/KT in [d,s] layout (RoPE fused on eviction),
  V via PE transpose -> [k,d] (+ones col) ->
  scores ST[k,q] = KT.T-slices @ QT (per head, causal block-skipped) ->
  ACT exp (scale=1/8) -> P^T -> PV: OT'[65,q] = [V|1].T @ P^T (Z row free) ->
  normalize via DVE reciprocal + K=1 ones-broadcast matmul ->
  out[q,e] = sum_m OTn_m.T @ WoT_m  -> partial out to DRAM.
"""
import math
import numpy as np

B, S, E = 2, 2048, 2048
NH, NKV, D = 32, 8, 64
G = 4                      # head groups
HL = NH // G               # 8 local q heads
KVL = NKV // G             # 2 local kv heads
ROPE_BASE = 10000.0
N_CORES = 8
PB = 512                   # projection s-band width
QB = 1024                  # attention q-band width
NPB = S // PB              # 4
NQB = S // QB              # 2
NC_E = E // 128            # 16 e-chunks

_RUNTIME = {}


def _build_nc():
    import concourse.bass as bass
    import concourse.tile as tile
    import concourse.mybir as mybir
    from concourse import bacc

    F32 = mybir.dt.float32
    F32R = mybir.dt.float32r
    AF = mybir.ActivationFunctionType

    nc = bacc.Bacc("TRN2", target_bir_lowering=False, debug=False,
                   num_devices=N_CORES)

    xT_d = nc.dram_tensor("xT", [E, S], F32R, kind="ExternalInput").ap()
    wq_d = nc.dram_tensor("wqT", [E, HL * D], F32R, kind="ExternalInput").ap()
    wk_d = nc.dram_tensor("wkT", [E, KVL * D], F32R, kind="ExternalInput").ap()
    wv_d = nc.dram_tensor("wvT", [E, KVL * D], F32R, kind="ExternalInput").ap()
    wo_d = nc.dram_tensor("woT", [HL * D, E], F32R, kind="ExternalInput").ap()
    cos_d = nc.dram_tensor("cosf", [128, S], F32, kind="ExternalInput").ap()
    sin_d = nc.dram_tensor("sinS", [128, S], F32, kind="ExternalInput").ap()
    msk_d = nc.dram_tensor("masks", [128, 4, PB], F32, kind="ExternalInput").ap()
    id_d = nc.dram_tensor("ident", [128, 64], F32, kind="ExternalInput").ap()
    ones_d = nc.dram_tensor("onesc", [128, 64], F32R, kind="ExternalInput").ap()
    out_d = nc.dram_tensor("out", [S, E], F32, kind="ExternalOutput").ap()

    with tile.TileContext(nc) as tc:
        with (
            tc.tile_pool(name="const", bufs=1) as const_p,
            tc.tile_pool(name="qts", bufs=1) as qt_p,
            tc.tile_pool(name="otn", bufs=1) as otn_p,
        ):
            # ---- constants / persistent tiles ----
            cosf = const_p.tile([128, S], F32, tag="cosf")
            sinS = const_p.tile([128, S], F32, tag="sinS")
            masks = const_p.tile([128, 4, PB], F32, tag="masks")
            ident = const_p.tile([128, 64], F32, tag="ident")
            onesc = const_p.tile([128, 64], F32R, tag="onesc")
            QT = [qt_p.tile([128, S], F32R, tag=f"qt{m}", name=f"qt{m}")
                  for m in range(4)]
            KT = qt_p.tile([128, S], F32R, tag="kt")
            # V' per kv head: 16 blocks of [128, 65] ([V | ones])
            Vp = [qt_p.tile([128, (S // 128) * 65], F32R, tag=f"vp{v}",
                            name=f"vp{v}") for v in range(KVL)]
            onesV = const_p.tile([128, S // 128], F32, tag="onesV")
            nc.vector.memset(onesV[:], 1.0)
            # prefire ACT exp table load during initial DMA
            warm = const_p.tile([1, 2], F32, tag="warm")
            nc.vector.memset(warm[:], 0.0)
            nc.scalar.activation(warm[:], warm[:], AF.Exp, scale=1.0)
            for v in range(KVL):
                vv = Vp[v].rearrange("p (t c) -> p t c", c=65)
                nc.vector.tensor_copy(vv[:, :, 64], onesV[:])

            OTn = [otn_p.tile([128, S], F32R, tag=f"otn{m}", name=f"otn{m}")
                   for m in range(4)]

            # ---------- phase 1: projections + RoPE + V transpose ----------
            with (
                tc.tile_pool(name="wts", bufs=1) as w_p,
                tc.tile_pool(name="xs", bufs=8) as x_p,
                tc.tile_pool(name="pjps", bufs=1, space="PSUM") as pj_ps,
                tc.tile_pool(name="vtps", bufs=2, space="PSUM") as vt_ps,
                tc.tile_pool(name="rope", bufs=2) as rope_p,
            ):
                wq_s = w_p.tile([128, NC_E, HL * D], F32R, tag="wq")
                wk_s = w_p.tile([128, NC_E, KVL * D], F32R, tag="wk")
                wv_s = w_p.tile([128, NC_E, KVL * D], F32R, tag="wv")
                wqr = wq_d.rearrange("(c p) m -> p c m", p=128)
                wkr = wk_d.rearrange("(c p) m -> p c m", p=128)
                wvr = wv_d.rearrange("(c p) m -> p c m", p=128)

                deferred_vtrans = []
                for j in range(NPB):
                    sb = PB * j
                    psQ = [pj_ps.tile([128, PB], F32, tag=f"psq{m}",
                                      name=f"psq{m}") for m in range(4)]
                    psK = pj_ps.tile([128, PB], F32, tag="psk")
                    psV = pj_ps.tile([128, PB], F32, tag="psv")
                    for c in range(NC_E):
                        xt = x_p.tile([128, PB], F32R, tag="xt")
                        nc.sync.dma_start(
                            xt[:], xT_d[128 * c:128 * (c + 1), sb:sb + PB])
                        if j == 0:
                            # stream weights chunk-wise alongside band 0
                            nc.sync.dma_start(wv_s[:, c, :], wvr[:, c, :])
                            nc.sync.dma_start(wk_s[:, c, :], wkr[:, c, :])
                            nc.sync.dma_start(wq_s[:, c, :], wqr[:, c, :])
                            if c == 11:
                                nc.sync.dma_start(cosf[:], cos_d[:])
                            elif c == 13:
                                nc.sync.dma_start(sinS[:], sin_d[:])
                            elif c == 15:
                                nc.sync.dma_start(ident[:], id_d[:])
                        st = (c == 0)
                        sp = (c == NC_E - 1)
                        # emission order matches eviction-free order
                        nc.tensor.matmul(psV[:], wv_s[:, c, :], xt[:],
                                         start=st, stop=sp)
                        nc.tensor.matmul(psK[:], wk_s[:, c, :], xt[:],
                                         start=st, stop=sp)
                        for m in (2, 0, 3, 1):
                            nc.tensor.matmul(
                                psQ[m][:], wq_s[:, c, 128 * m:128 * (m + 1)],
                                xt[:], start=st, stop=sp)
                    if j == 1:
                        nc.sync.dma_start(masks[:], msk_d[:])
                        nc.sync.dma_start(onesc[:], ones_d[:])
                    # evict psum banks: each bank has exactly ONE reader
                    # (a copy op), split across DVE and ACT
                    vtmp = rope_p.tile([128, PB], F32, tag="vtmp")
                    nc.vector.tensor_copy(vtmp[:], psV[:])
                    banks = [(psK, KT, "act"), (psQ[2], QT[2], "dve"),
                             (psQ[0], QT[0], "act"), (psQ[3], QT[3], "dve"),
                             (psQ[1], QT[1], "act")]
                    qtb = rope_p.tile([128, 5 * PB], F32, tag="qtb", bufs=1)
                    for i, (bank, _dst, eng) in enumerate(banks):
                        sl = qtb[:, PB * i:PB * (i + 1)]
                        if eng == "act":
                            nc.scalar.copy(sl, bank[:])
                        else:
                            nc.vector.tensor_copy(sl, bank[:])
                    t1s = []
                    for i in range(len(banks)):
                        t1 = rope_p.tile([128, PB], F32, tag="t1", bufs=5)
                        nc.vector.tensor_mul(t1[:], qtb[:, PB * i:PB * (i + 1)],
                                             cosf[:, sb:sb + PB])
                        t1s.append(t1)
                    swb = rope_p.tile([128, 5 * PB], F32, tag="swb", bufs=1)
                    for blk in range(4):
                        srcb = blk ^ 1
                        nc.sync.dma_start(
                            swb[32 * blk:32 * (blk + 1), :],
                            qtb[32 * srcb:32 * (srcb + 1), :])
                    for i, (bank, dst, _eng) in enumerate(banks):
                        t2 = rope_p.tile([128, PB], F32, tag="t2")
                        nc.vector.tensor_mul(t2[:], swb[:, PB * i:PB * (i + 1)],
                                             sinS[:, sb:sb + PB])
                        nc.vector.tensor_add(dst[:, sb:sb + PB],
                                             t1s[i][:], t2[:])
                    # V: psV [2kv*64, s] -> per-head [s, 64] blocks + ones col
                    def vtrans(j=j, vtmp=vtmp, pool=None):
                        for v in range(KVL):
                            for tl in range(PB // 128):
                                tg = (PB // 128) * j + tl
                                pst = (pool or vt_ps).tile(
                                    [128, 64], F32,
                                    tag="vtr" if pool is None else "ot",
                                    name="pst")
                                nc.tensor.transpose(
                                    pst[:], vtmp[64 * v:64 * (v + 1),
                                                 128 * tl:128 * (tl + 1)],
                                    ident[64 * v:64 * (v + 1), :])
                                nc.vector.tensor_copy(
                                    Vp[v][:, 65 * tg:65 * tg + 64], pst[:])
                    if j < NPB - 1:
                        vtrans()
                    else:
                        deferred_vtrans.append(vtrans)

            # ---------- phase 2+3: attention with interleaved out-proj ----
            with (
                tc.tile_pool(name="scps", bufs=2, space="PSUM") as sc_ps,
                tc.tile_pool(name="otps", bufs=2, space="PSUM") as ot_ps,
                tc.tile_pool(name="pts", bufs=4) as pt_p,
                tc.tile_pool(name="nrm", bufs=2) as nm_p,
                tc.tile_pool(name="wo", bufs=1) as wo_p,
                tc.tile_pool(name="sos", bufs=3) as so_p,
            ):
                wo_s = wo_p.tile([128, 4, E], F32R, tag="wo")
                wor = wo_d.rearrange("(m p) e -> p m e", p=128)
                for m in range(4):
                    nc.sync.dma_start(wo_s[:, m, :], wor[:, m, :])

                def out_proj_qt(qt, act_copy=False):
                    # two [128,512] accumulation groups per sc-ring slot
                    for eb2 in range(2):
                        po = ot_ps.tile([128, QB], F32, tag="ot", name="po")
                        for half in range(2):
                            eb = 2 * eb2 + half
                            for m in range(4):
                                nc.tensor.matmul(
                                    po[:, 512 * half:512 * (half + 1)],
                                    OTn[m][:, 128 * qt:128 * (qt + 1)],
                                    wo_s[:, m, 512 * eb:512 * (eb + 1)],
                                    start=(m == 0), stop=(m == 3))
                        so = so_p.tile([128, QB], F32, tag="so")
                        if act_copy:
                            nc.scalar.copy(so[:], po[:])
                        else:
                            nc.vector.tensor_copy(so[:], po[:])
                        nc.sync.dma_start(
                            out_d[128 * qt:128 * (qt + 1),
                                  1024 * eb2:1024 * (eb2 + 1)], so[:])

                def attn_head(j, h):
                    q0 = QB * j
                    ntile = (QB // 128) * (j + 1)
                    nL = ntile - 4
                    v = h // 4
                    m = h % 4
                    base = 64 * v
                    ot = ot_ps.tile([65, QB], F32, tag="ot")
                    rh0 = ntile - 4          # first right-only tile
                    from collections import deque
                    pend = deque()           # deferred PV emitters (2 deep)

                    def trim(mi):
                        # first live column (rounded so matmul N >= 256)
                        if mi <= 0:
                            return 0
                        return 128 if mi == 1 else 256

                    for t in range(ntile):
                        r = t - (QB // 128) * j
                        if r >= 4 and (t - rh0) % 2 == 1:
                            continue         # handled with its pair below
                        ks = KT[base:base + 64, 128 * t:128 * (t + 1)]
                        sc = sc_ps.tile([128, QB], F32, tag="sc")
                        pt = pt_p.tile([128, QB], F32R, tag="pt", bufs=4)
                        vs = Vp[v][:, 65 * t:65 * t + 65]
                        if r < 0:       # fully causal tile
                            nc.tensor.matmul(
                                sc[:, 0:512], ks,
                                QT[m][base:base + 64, q0:q0 + 512],
                                start=True, stop=True)
                            nc.tensor.matmul(
                                sc[:, 512:1024], ks,
                                QT[m][base:base + 64, q0 + 512:q0 + 1024],
                                start=True, stop=True)
                            nc.scalar.activation(pt[:], sc[:], AF.Exp,
                                                 scale=0.125)

                            def mk_pv(t=t, vs=vs, pt=pt):
                                nc.tensor.matmul(
                                    ot[:, 0:512], vs, pt[:, 0:512],
                                    start=(t == 0), stop=(t == nL - 1))
                                nc.tensor.matmul(
                                    ot[:, 512:1024], vs, pt[:, 512:1024],
                                    start=(t == 0), stop=(t == ntile - 1))
                        elif r < 4:     # straddles left half
                            ts_ = trim(r)
                            nc.tensor.matmul(
                                sc[:, ts_:512], ks,
                                QT[m][base:base + 64,
                                      q0 + ts_:q0 + 512],
                                start=True, stop=True)
                            nc.tensor.matmul(
                                sc[:, 512:1024], ks,
                                QT[m][base:base + 64, q0 + 512:q0 + 1024],
                                start=True, stop=True)
                            nc.scalar.activation(pt[:, ts_:1024],
                                                 sc[:, ts_:1024], AF.Exp,
                                                 scale=0.125)
                            ptm = pt_p.tile([128, 512], F32R, tag="ptm")
                            nc.vector.tensor_mul(ptm[:, ts_:512],
                                                 pt[:, ts_:512],
                                                 masks[:, r, ts_:512])

                            def mk_pv(t=t, vs=vs, pt=pt, ptm=ptm, ts_=ts_):
                                nc.tensor.matmul(
                                    ot[:, ts_:512], vs, ptm[:, ts_:512],
                                    start=(t == 0), stop=(t == nL - 1))
                                nc.tensor.matmul(
                                    ot[:, 512:1024], vs, pt[:, 512:1024],
                                    start=(t == 0), stop=(t == ntile - 1))
                        else:           # right-only tiles, merged in pairs
                            t2_ = t + 1
                            tsA = trim(r - 4)
                            tsB = trim(r - 3)
                            ks2 = KT[base:base + 64,
                                     128 * t2_:128 * (t2_ + 1)]
                            vs2 = Vp[v][:, 65 * t2_:65 * t2_ + 65]
                            nc.tensor.matmul(
                                sc[:, tsA:512], ks,
                                QT[m][base:base + 64,
                                      q0 + 512 + tsA:q0 + 1024],
                                start=True, stop=True)
                            nc.tensor.matmul(
                                sc[:, 512 + tsB:1024], ks2,
                                QT[m][base:base + 64,
                                      q0 + 512 + tsB:q0 + 1024],
                                start=True, stop=True)
                            if tsB == 0:
                                nc.scalar.activation(pt[:, tsA:1024],
                                                     sc[:, tsA:1024],
                                                     AF.Exp, scale=0.125)
                            else:
                                nc.scalar.activation(pt[:, tsA:512],
                                                     sc[:, tsA:512],
                                                     AF.Exp, scale=0.125)
                                nc.scalar.activation(pt[:, 512 + tsB:1024],
                                                     sc[:, 512 + tsB:1024],
                                                     AF.Exp, scale=0.125)
                            ptm = pt_p.tile([128, 512], F32R, tag="ptm")
                            nc.vector.tensor_mul(ptm[:, tsA:512],
                                                 pt[:, tsA:512],
                                                 masks[:, r - 4, tsA:512])
                            ptm2 = pt_p.tile([128, 512], F32R, tag="ptm")
                            nc.vector.tensor_mul(ptm2[:, tsB:512],
                                                 pt[:, 512 + tsB:1024],
                                                 masks[:, r - 3, tsB:512])

                            def mk_pv(t=t, t2_=t2_, vs=vs, vs2=vs2,
                                      ptm=ptm, ptm2=ptm2, tsA=tsA, tsB=tsB):
                                nc.tensor.matmul(
                                    ot[:, 512 + tsA:1024], vs,
                                    ptm[:, tsA:512],
                                    start=(t == 0), stop=False)
                                nc.tensor.matmul(
                                    ot[:, 512 + tsB:1024], vs2,
                                    ptm2[:, tsB:512],
                                    start=False, stop=(t2_ == ntile - 1))
                        pend.append(mk_pv)
                        if len(pend) > 2:
                            pend.popleft()()
                    while pend:
                        pend.popleft()()
                    # normalize: rows 0..63 /= row 64
                    r1 = nm_p.tile([65, QB], F32R, tag="r1")
                    with nc.allow_low_precision(reason="fp32r softmax recip"):
                        nc.vector.reciprocal(r1[64:65, :], ot[64:65, :])
                    rbt = ot_ps.tile([65, QB], F32, tag="ot", name="rbt")
                    rb = rbt[0:64, :]
                    nc.tensor.matmul(rb[:, 0:512], onesc[64:65, :],
                                     r1[64:65, 0:512], start=True, stop=True)
                    nc.tensor.matmul(rb[:, 512:1024], onesc[64:65, :],
                                     r1[64:65, 512:1024],
                                     start=True, stop=True)
                    rbs = nm_p.tile([64, QB], F32, tag="rbs")
                    nc.vector.tensor_copy(rbs[:], rb[:])
                    if v == 0:
                        nc.vector.tensor_mul(OTn[m][0:64, q0:q0 + QB],
                                             ot[0:64, :], rbs[:])
                    else:
                        stg = nm_p.tile([64, QB], F32R, tag="stg")
                        nc.vector.tensor_mul(stg[:], ot[0:64, :], rbs[:])
                        nc.sync.dma_start(OTn[m][64:128, q0:q0 + QB],
                                          stg[:])

                for h in range(HL):
                    attn_head(0, h)
                    if h == 0:
                        for fn in deferred_vtrans:
                            fn(pool=ot_ps)
                for h in range(HL):
                    attn_head(1, h)
                    if h > 0:
                        # previous head's rows: deps resolved, fills PE slack
                        out_proj_qt(h - 1)
                out_proj_qt(HL - 1)
                for qt in range(QB // 128, S // 128):
                    out_proj_qt(qt, act_copy=True)

    nc.compile()
    return nc


class _Runner:
    """Persistent jitted SPMD executor (mirrors bass2jax.run_bass_via_pjrt)."""

    def __init__(self, nc):
        import jax
        import jax.core
        import concourse.mybir as mybir
        from concourse import bass2jax
        from jax.experimental.shard_map import shard_map
        from jax.sharding import Mesh, PartitionSpec

        bass2jax.install_neuronx_cc_hook()
        self.nc = nc
        in_names, out_names, out_avals, zero_outs = [], [], [], []
        part_name = (nc.partition_id_tensor.name
                     if nc.partition_id_tensor is not None else None)
        for alloc in nc.m.functions[0].allocations:
            if not isinstance(alloc, mybir.MemoryLocationSet):
                continue
            name = alloc.memorylocations[0].name
            if alloc.kind == "ExternalInput":
                if name != part_name:
                    in_names.append(name)
            elif alloc.kind == "ExternalOutput":
                out_names.append(name)
                shape = tuple(alloc.tensor_shape)
                dtype = mybir.dt.np(alloc.dtype)
                out_avals.append(jax.core.ShapedArray(shape, dtype))
                zero_outs.append(np.zeros(shape, dtype))
        self.in_names = list(in_names)
        self.out_names = out_names
        self.out_avals = out_avals
        self.zero_outs = zero_outs
        bind_in_names = tuple(
            in_names + out_names + ([part_name] if part_name else []))

        def _body(*args):
            operands = list(args)
            if part_name is not None:
                operands.append(bass2jax.partition_id_tensor())
            outs = bass2jax._bass_exec_p.bind(
                *operands,
                out_avals=tuple(out_avals),
                in_names=bind_in_names,
                out_names=tuple(out_names),
                lowering_input_output_aliases=(),
                sim_require_finite=True,
                sim_require_nnan=True,
                nc=nc,
            )
            return tuple(outs)

        self._body = _body
        devices = jax.devices()[:N_CORES]
        assert len(devices) == N_CORES, f"need {N_CORES} cores, saw {len(devices)}"
        mesh = Mesh(np.asarray(devices), ("core",))
        nin = len(in_names) + len(out_names)
        self._fn = jax.jit(
            shard_map(_body, mesh=mesh,
                      in_specs=(PartitionSpec("core"),) * nin,
                      out_specs=(PartitionSpec("core"),) * len(out_names),
                      check_rep=False),
            keep_unused=True,
        )
        self._mesh = mesh
        self._shard_map = shard_map
        self._PartitionSpec = PartitionSpec
        self._jax = jax

    def make_chain(self, k):
        """jit of k dependency-chained kernel executions (for timing)."""
        jax = self._jax
        n_in = len(self.in_names)
        n_out = len(self.out_names)

        def chain(*args):
            ins = list(args[:n_in])
            zs = list(args[n_in:])
            for _ in range(k):
                zs = list(self._body(*ins, *zs))
            return tuple(zs)

        nin = n_in + n_out
        return jax.jit(
            self._shard_map(chain, mesh=self._mesh,
                            in_specs=(self._PartitionSpec("core"),) * nin,
                            out_specs=(self._PartitionSpec("core"),) * n_out,
                            check_rep=False),
            keep_unused=True,
        )

    def prepare(self, in_maps):
        """Concatenate per-core inputs and place on device."""
        jax = self._jax
        concat = [
            np.concatenate([np.asarray(in_maps[c][n]) for c in range(N_CORES)],
                           axis=0)
            for n in self.in_names
        ]
        concat += [
            np.zeros((N_CORES * z.shape[0], *z.shape[1:]), z.dtype)
            for z in self.zero_outs
        ]
        return [jax.device_put(a) for a in concat]

    def execute(self, dev_args):
        out_arrs = self._fn(*dev_args)
        self._jax.block_until_ready(out_arrs)
        return out_arrs

    def run(self, in_maps):
        out_arrs = self.execute(self.prepare(in_maps))
        return [
            {n: np.asarray(out_arrs[i]).reshape(N_CORES, *self.out_avals[i].shape)[c]
             for i, n in enumerate(self.out_names)}
            for c in range(N_CORES)
        ]


def _head_perm():
    """Local-head row permutation: chunk m holds heads (m, m+4)."""
    order = []
    for m in range(4):
        order += list(range(64 * m, 64 * m + 64))
        order += list(range(64 * (m + 4), 64 * (m + 4) + 64))
    return np.array(order)


def prep_inputs(x, position_ids, Wq, Wk, Wv, Wo):
    x = np.asarray(x, dtype=np.float32)
    pos = np.asarray(position_ids).astype(np.float32)
    Wq = np.asarray(Wq, dtype=np.float32)
    Wk = np.asarray(Wk, dtype=np.float32)
    Wv = np.asarray(Wv, dtype=np.float32)
    Wo = np.asarray(Wo, dtype=np.float32)

    inv_freq = (1.0 / (ROPE_BASE ** (np.arange(0, D, 2, dtype=np.float32) / D))
                ).astype(np.float32)
    ang = pos[:, None] * inv_freq[None, :]          # (S, 32) fp32
    cosa = np.cos(ang).astype(np.float32)           # (S, 32)
    sina = np.sin(ang).astype(np.float32)
    ridx = np.arange(128)
    cosf = np.ascontiguousarray(cosa[:, ridx % 32].T)          # (128, S)
    sgn = np.where((ridx % 64) < 32, -1.0, 1.0).astype(np.float32)
    sinS = np.ascontiguousarray(sina[:, ridx % 32].T * sgn[:, None])

    kk = np.arange(128)[:, None]
    qq = np.arange(PB)[None, :]
    masks = np.stack([(kk <= qq - 128 * mm).astype(np.float32)
                      for mm in range(4)], axis=1)             # (128, 4, 512)
    ident = np.tile(np.eye(64, dtype=np.float32), (2, 1))      # (128, 64)
    onesc = np.ones((128, 64), dtype=np.float32)

    perm = _head_perm()
    in_maps = []
    for i in range(N_CORES):
        g, b = i // 2, i % 2
        xT = np.ascontiguousarray(x[b].T)                      # (E, S)
        wq_g = Wq[512 * g:512 * (g + 1)][perm]                 # (512, E)
        wqT = np.ascontiguousarray(wq_g.T)                     # (E, 512)
        wkT = np.ascontiguousarray(Wk[128 * g:128 * (g + 1)].T)
        wvT = np.ascontiguousarray(Wv[128 * g:128 * (g + 1)].T)
        wo_g = Wo[:, 512 * g:512 * (g + 1)][:, perm]           # (E, 512)
        woT = np.ascontiguousarray(wo_g.T)                     # (512, E)
        in_maps.append({
            "xT": xT, "wqT": wqT, "wkT": wkT, "wvT": wvT, "woT": woT,
            "cosf": cosf, "sinS": sinS, "masks": masks,
            "ident": ident, "onesc": onesc,
        })
    return in_maps


def get_runner():
    if "runner" not in _RUNTIME:
        nc = _build_nc()
        _RUNTIME["runner"] = _Runner(nc)
    return _RUNTIME["runner"]


def assemble(results):
    out = np.empty((B, S, E), dtype=np.float32)
    for b in range(B):
        acc = results[b]["out"].copy()
        for g in range(1, G):
            acc += results[2 * g + b]["out"]
        out[b] = acc
    return out


def kernel(x, position_ids, Wq, Wk, Wv, Wo):
    runner = get_runner()
    in_maps = prep_inputs(x, position_ids, Wq, Wk, Wv, Wo)
    results = runner.run(in_maps)
    return assemble(results)


# revision 59
# speedup vs baseline: 1.0075x; 1.0075x over previous
"""TRN2 Bass kernel for nn_AttentionBlock (B=2,S=2048,E=2048,H=32,KV=8,D=64).

Sharding: 8 cores = 4 head-groups x 2 batch. Each core computes 8 q-heads /
2 kv-heads of one batch element, producing a partial (S,E) output; host sums
the 4 group partials per batch.

Per-core dataflow (all matmuls in fp32r: 1 cycle/row at N>=512):
  xT (E,S) -> [proj] -> QT/KT in [d,s] layout (RoPE fused on eviction),
  V via PE transpose -> [k,d] (+ones col) ->
  scores ST[k,q] = KT.T-slices @ QT (per head, causal block-skipped) ->
  ACT exp (scale=1/8) -> P^T -> PV: OT'[65,q] = [V|1].T @ P^T (Z row free) ->
  normalize via DVE reciprocal + K=1 ones-broadcast matmul ->
  out[q,e] = sum_m OTn_m.T @ WoT_m  -> partial out to DRAM.
"""
import math
import numpy as np

B, S, E = 2, 2048, 2048
NH, NKV, D = 32, 8, 64
G = 4                      # head groups
HL = NH // G               # 8 local q heads
KVL = NKV // G             # 2 local kv heads
ROPE_BASE = 10000.0
N_CORES = 8
PB = 512                   # projection s-band width
QB = 1024                  # attention q-band width
NPB = S // PB              # 4
NQB = S // QB              # 2
NC_E = E // 128            # 16 e-chunks

_RUNTIME = {}


def _build_nc():
    import concourse.bass as bass
    import concourse.tile as tile
    import concourse.mybir as mybir
    from concourse import bacc

    F32 = mybir.dt.float32
    F32R = mybir.dt.float32r
    AF = mybir.ActivationFunctionType

    nc = bacc.Bacc("TRN2", target_bir_lowering=False, debug=False,
                   num_devices=N_CORES)

    xT_d = nc.dram_tensor("xT", [E, S], F32R, kind="ExternalInput").ap()
    wq_d = nc.dram_tensor("wqT", [E, HL * D], F32R, kind="ExternalInput").ap()
    wk_d = nc.dram_tensor("wkT", [E, KVL * D], F32R, kind="ExternalInput").ap()
    wv_d = nc.dram_tensor("wvT", [E, KVL * D], F32R, kind="ExternalInput").ap()
    wo_d = nc.dram_tensor("woT", [HL * D, E], F32R, kind="ExternalInput").ap()
    cos_d = nc.dram_tensor("cosf", [128, S], F32, kind="ExternalInput").ap()
    sin_d = nc.dram_tensor("sinS", [128, S], F32, kind="ExternalInput").ap()
    msk_d = nc.dram_tensor("masks", [128, 4, PB], F32, kind="ExternalInput").ap()
    id_d = nc.dram_tensor("ident", [128, 64], F32, kind="ExternalInput").ap()
    ones_d = nc.dram_tensor("onesc", [128, 64], F32R, kind="ExternalInput").ap()
    out_d = nc.dram_tensor("out", [S, E], F32, kind="ExternalOutput").ap()

    with tile.TileContext(nc) as tc:
        with (
            tc.tile_pool(name="const", bufs=1) as const_p,
            tc.tile_pool(name="qts", bufs=1) as qt_p,
            tc.tile_pool(name="otn", bufs=1) as otn_p,
        ):
            # ---- constants / persistent tiles ----
            cosf = const_p.tile([128, S], F32, tag="cosf")
            sinS = const_p.tile([128, S], F32, tag="sinS")
            masks = const_p.tile([128, 4, PB], F32, tag="masks")
            ident = const_p.tile([128, 64], F32, tag="ident")
            onesc = const_p.tile([128, 64], F32R, tag="onesc")
            QT = [qt_p.tile([128, S], F32R, tag=f"qt{m}", name=f"qt{m}")
                  for m in range(4)]
            KT = qt_p.tile([128, S], F32R, tag="kt")
            # V' per kv head: 16 blocks of [128, 65] ([V | ones])
            Vp = [qt_p.tile([128, (S // 128) * 65], F32R, tag=f"vp{v}",
                            name=f"vp{v}") for v in range(KVL)]
            onesV = const_p.tile([128, S // 128], F32, tag="onesV")
            nc.vector.memset(onesV[:], 1.0)
            # prefire ACT exp table load during initial DMA
            warm = const_p.tile([1, 2], F32, tag="warm")
            nc.vector.memset(warm[:], 0.0)
            nc.scalar.activation(warm[:], warm[:], AF.Exp, scale=1.0)
            for v in range(KVL):
                vv = Vp[v].rearrange("p (t c) -> p t c", c=65)
                nc.vector.tensor_copy(vv[:, :, 64], onesV[:])

            OTn = [otn_p.tile([128, S], F32R, tag=f"otn{m}", name=f"otn{m}")
                   for m in range(4)]

            # ---------- phase 1: projections + RoPE + V transpose ----------
            with (
                tc.tile_pool(name="wts", bufs=1) as w_p,
                tc.tile_pool(name="xs", bufs=8) as x_p,
                tc.tile_pool(name="pjps", bufs=1, space="PSUM") as pj_ps,
                tc.tile_pool(name="vtps", bufs=2, space="PSUM") as vt_ps,
                tc.tile_pool(name="rope", bufs=2) as rope_p,
            ):
                wq_s = w_p.tile([128, NC_E, HL * D], F32R, tag="wq")
                wk_s = w_p.tile([128, NC_E, KVL * D], F32R, tag="wk")
                wv_s = w_p.tile([128, NC_E, KVL * D], F32R, tag="wv")
                wqr = wq_d.rearrange("(c p) m -> p c m", p=128)
                wkr = wk_d.rearrange("(c p) m -> p c m", p=128)
                wvr = wv_d.rearrange("(c p) m -> p c m", p=128)

                deferred_vtrans = []
                for j in range(NPB):
                    sb = PB * j
                    psQ = [pj_ps.tile([128, PB], F32, tag=f"psq{m}",
                                      name=f"psq{m}") for m in range(4)]
                    psK = pj_ps.tile([128, PB], F32, tag="psk")
                    psV = pj_ps.tile([128, PB], F32, tag="psv")
                    for c in range(NC_E):
                        xt = x_p.tile([128, PB], F32R, tag="xt")
                        nc.sync.dma_start(
                            xt[:], xT_d[128 * c:128 * (c + 1), sb:sb + PB])
                        if j == 0:
                            # stream weights chunk-wise alongside band 0
                            nc.sync.dma_start(wv_s[:, c, :], wvr[:, c, :])
                            nc.sync.dma_start(wk_s[:, c, :], wkr[:, c, :])
                            nc.sync.dma_start(wq_s[:, c, :], wqr[:, c, :])
                            if c == 11:
                                nc.sync.dma_start(cosf[:], cos_d[:])
                            elif c == 13:
                                nc.sync.dma_start(sinS[:], sin_d[:])
                            elif c == 15:
                                nc.sync.dma_start(ident[:], id_d[:])
                        st = (c == 0)
                        sp = (c == NC_E - 1)
                        # emission order matches eviction-free order
                        nc.tensor.matmul(psV[:], wv_s[:, c, :], xt[:],
                                         start=st, stop=sp)
                        nc.tensor.matmul(psK[:], wk_s[:, c, :], xt[:],
                                         start=st, stop=sp)
                        for m in (2, 0, 3, 1):
                            nc.tensor.matmul(
                                psQ[m][:], wq_s[:, c, 128 * m:128 * (m + 1)],
                                xt[:], start=st, stop=sp)
                    if j == 1:
                        nc.sync.dma_start(masks[:], msk_d[:])
                        nc.sync.dma_start(onesc[:], ones_d[:])
                    # evict psum banks: each bank has exactly ONE reader
                    # (a copy op), split across DVE and ACT
                    vtmp = rope_p.tile([128, PB], F32, tag="vtmp")
                    nc.vector.tensor_copy(vtmp[:], psV[:])
                    banks = [(psK, KT, "act"), (psQ[2], QT[2], "dve"),
                             (psQ[0], QT[0], "act"), (psQ[3], QT[3], "dve"),
                             (psQ[1], QT[1], "act")]
                    qtb = rope_p.tile([128, 5 * PB], F32, tag="qtb", bufs=1)
                    for i, (bank, _dst, eng) in enumerate(banks):
                        sl = qtb[:, PB * i:PB * (i + 1)]
                        if eng == "act":
                            nc.scalar.copy(sl, bank[:])
                        else:
                            nc.vector.tensor_copy(sl, bank[:])
                    t1s = []
                    for i in range(len(banks)):
                        t1 = rope_p.tile([128, PB], F32, tag="t1", bufs=5)
                        nc.vector.tensor_mul(t1[:], qtb[:, PB * i:PB * (i + 1)],
                                             cosf[:, sb:sb + PB])
                        t1s.append(t1)
                    swb = rope_p.tile([128, 5 * PB], F32, tag="swb", bufs=1)
                    for blk in range(4):
                        srcb = blk ^ 1
                        nc.sync.dma_start(
                            swb[32 * blk:32 * (blk + 1), :],
                            qtb[32 * srcb:32 * (srcb + 1), :])
                    for i, (bank, dst, _eng) in enumerate(banks):
                        t2 = rope_p.tile([128, PB], F32, tag="t2")
                        nc.vector.tensor_mul(t2[:], swb[:, PB * i:PB * (i + 1)],
                                             sinS[:, sb:sb + PB])
                        nc.vector.tensor_add(dst[:, sb:sb + PB],
                                             t1s[i][:], t2[:])
                    # V: psV [2kv*64, s] -> per-head [s, 64] blocks + ones col
                    def vtrans(j=j, vtmp=vtmp, pool=None):
                        for v in range(KVL):
                            for tl in range(PB // 128):
                                tg = (PB // 128) * j + tl
                                pst = (pool or vt_ps).tile(
                                    [128, 64], F32,
                                    tag="vtr" if pool is None else "ot",
                                    name="pst")
                                nc.tensor.transpose(
                                    pst[:], vtmp[64 * v:64 * (v + 1),
                                                 128 * tl:128 * (tl + 1)],
                                    ident[64 * v:64 * (v + 1), :])
                                nc.vector.tensor_copy(
                                    Vp[v][:, 65 * tg:65 * tg + 64], pst[:])
                    if j < NPB - 1:
                        vtrans()
                    else:
                        deferred_vtrans.append(vtrans)

            # ---------- phase 2+3: attention with interleaved out-proj ----
            with (
                tc.tile_pool(name="scps", bufs=2, space="PSUM") as sc_ps,
                tc.tile_pool(name="otps", bufs=2, space="PSUM") as ot_ps,
                tc.tile_pool(name="pts", bufs=4) as pt_p,
                tc.tile_pool(name="nrm", bufs=2) as nm_p,
                tc.tile_pool(name="wo", bufs=1) as wo_p,
                tc.tile_pool(name="sos", bufs=3) as so_p,
            ):
                wo_s = wo_p.tile([128, 4, E], F32R, tag="wo")
                wor = wo_d.rearrange("(m p) e -> p m e", p=128)
                for m in range(4):
                    nc.sync.dma_start(wo_s[:, m, :], wor[:, m, :])

                def out_proj_qt(qt, act_copy=False):
                    # two [128,512] accumulation groups per sc-ring slot
                    for eb2 in range(2):
                        po = ot_ps.tile([128, QB], F32, tag="ot", name="po")
                        for half in range(2):
                            eb = 2 * eb2 + half
                            for m in range(4):
                                nc.tensor.matmul(
                                    po[:, 512 * half:512 * (half + 1)],
                                    OTn[m][:, 128 * qt:128 * (qt + 1)],
                                    wo_s[:, m, 512 * eb:512 * (eb + 1)],
                                    start=(m == 0), stop=(m == 3))
                        so = so_p.tile([128, QB], F32, tag="so")
                        if act_copy:
                            nc.scalar.copy(so[:], po[:])
                        else:
                            nc.vector.tensor_copy(so[:], po[:])
                        nc.sync.dma_start(
                            out_d[128 * qt:128 * (qt + 1),
                                  1024 * eb2:1024 * (eb2 + 1)], so[:])

                def attn_head(j, h):
                    q0 = QB * j
                    ntile = (QB // 128) * (j + 1)
                    nL = ntile - 4
                    v = h // 4
                    m = h % 4
                    base = 64 * v
                    ot = ot_ps.tile([65, QB], F32, tag="ot")
                    rh0 = ntile - 4          # first right-only tile
                    from collections import deque
                    pend = deque()           # deferred PV emitters (2 deep)

                    def trim(mi):
                        # first live column (rounded so matmul N >= 256)
                        if mi <= 0:
                            return 0
                        return 128 if mi == 1 else 256

                    for t in range(ntile):
                        r = t - (QB // 128) * j
                        if r >= 4 and (t - rh0) % 2 == 1:
                            continue         # handled with its pair below
                        ks = KT[base:base + 64, 128 * t:128 * (t + 1)]
                        sc = sc_ps.tile([128, QB], F32, tag="sc")
                        pt = pt_p.tile([128, QB], F32R, tag="pt", bufs=4)
                        vs = Vp[v][:, 65 * t:65 * t + 65]
                        if r < 0:       # fully causal tile
                            nc.tensor.matmul(
                                sc[:, 0:512], ks,
                                QT[m][base:base + 64, q0:q0 + 512],
                                start=True, stop=True)
                            nc.tensor.matmul(
                                sc[:, 512:1024], ks,
                                QT[m][base:base + 64, q0 + 512:q0 + 1024],
                                start=True, stop=True)
                            nc.scalar.activation(pt[:], sc[:], AF.Exp,
                                                 scale=0.125)

                            def mk_pv(t=t, vs=vs, pt=pt):
                                nc.tensor.matmul(
                                    ot[:, 0:512], vs, pt[:, 0:512],
                                    start=(t == 0), stop=(t == nL - 1))
                                nc.tensor.matmul(
                                    ot[:, 512:1024], vs, pt[:, 512:1024],
                                    start=(t == 0), stop=(t == ntile - 1))
                        elif r < 4:     # straddles left half
                            ts_ = trim(r)
                            nc.tensor.matmul(
                                sc[:, ts_:512], ks,
                                QT[m][base:base + 64,
                                      q0 + ts_:q0 + 512],
                                start=True, stop=True)
                            nc.tensor.matmul(
                                sc[:, 512:1024], ks,
                                QT[m][base:base + 64, q0 + 512:q0 + 1024],
                                start=True, stop=True)
                            nc.scalar.activation(pt[:, ts_:1024],
                                                 sc[:, ts_:1024], AF.Exp,
                                                 scale=0.125)
                            ptm = pt_p.tile([128, 512], F32R, tag="ptm")
                            nc.vector.tensor_mul(ptm[:, ts_:512],
                                                 pt[:, ts_:512],
                                                 masks[:, r, ts_:512])

                            def mk_pv(t=t, vs=vs, pt=pt, ptm=ptm, ts_=ts_):
                                nc.tensor.matmul(
                                    ot[:, ts_:512], vs, ptm[:, ts_:512],
                                    start=(t == 0), stop=(t == nL - 1))
                                nc.tensor.matmul(
                                    ot[:, 512:1024], vs, pt[:, 512:1024],
                                    start=(t == 0), stop=(t == ntile - 1))
                        else:           # right-only tiles, merged in pairs
                            t2_ = t + 1
                            tsA = trim(r - 4)
                            tsB = trim(r - 3)
                            ks2 = KT[base:base + 64,
                                     128 * t2_:128 * (t2_ + 1)]
                            vs2 = Vp[v][:, 65 * t2_:65 * t2_ + 65]
                            nc.tensor.matmul(
                                sc[:, tsA:512], ks,
                                QT[m][base:base + 64,
                                      q0 + 512 + tsA:q0 + 1024],
                                start=True, stop=True)
                            nc.tensor.matmul(
                                sc[:, 512 + tsB:1024], ks2,
                                QT[m][base:base + 64,
                                      q0 + 512 + tsB:q0 + 1024],
                                start=True, stop=True)
                            if tsB == 0:
                                nc.scalar.activation(pt[:, tsA:1024],
                                                     sc[:, tsA:1024],
                                                     AF.Exp, scale=0.125)
                            else:
                                nc.scalar.activation(pt[:, tsA:512],
                                                     sc[:, tsA:512],
                                                     AF.Exp, scale=0.125)
                                nc.scalar.activation(pt[:, 512 + tsB:1024],
                                                     sc[:, 512 + tsB:1024],
                                                     AF.Exp, scale=0.125)
                            ptm = pt_p.tile([128, 512], F32R, tag="ptm")
                            nc.vector.tensor_mul(ptm[:, tsA:512],
                                                 pt[:, tsA:512],
                                                 masks[:, r - 4, tsA:512])
                            ptm2 = pt_p.tile([128, 512], F32R, tag="ptm")
                            nc.vector.tensor_mul(ptm2[:, tsB:512],
                                                 pt[:, 512 + tsB:1024],
                                                 masks[:, r - 3, tsB:512])

                            def mk_pv(t=t, t2_=t2_, vs=vs, vs2=vs2,
                                      ptm=ptm, ptm2=ptm2, tsA=tsA, tsB=tsB):
                                nc.tensor.matmul(
                                    ot[:, 512 + tsA:1024], vs,
                                    ptm[:, tsA:512],
                                    start=(t == 0), stop=False)
                                nc.tensor.matmul(
                                    ot[:, 512 + tsB:1024], vs2,
                                    ptm2[:, tsB:512],
                                    start=False, stop=(t2_ == ntile - 1))
                        pend.append(mk_pv)
                        if len(pend) > 2:
                            pend.popleft()()
                    while pend:
                        pend.popleft()()
                    # normalize: rows 0..63 /= row 64
                    r1 = nm_p.tile([65, QB], F32R, tag="r1")
                    with nc.allow_low_precision(reason="fp32r softmax recip"):
                        nc.vector.reciprocal(r1[64:65, :], ot[64:65, :])
                    rbt = ot_ps.tile([65, QB], F32, tag="ot", name="rbt")
                    rb = rbt[0:64, :]
                    nc.tensor.matmul(rb[:, 0:512], onesc[64:65, :],
                                     r1[64:65, 0:512], start=True, stop=True)
                    nc.tensor.matmul(rb[:, 512:1024], onesc[64:65, :],
                                     r1[64:65, 512:1024],
                                     start=True, stop=True)
                    rbs = nm_p.tile([64, QB], F32, tag="rbs")
                    nc.vector.tensor_copy(rbs[:], rb[:])
                    if v == 0:
                        nc.vector.tensor_mul(OTn[m][0:64, q0:q0 + QB],
                                             ot[0:64, :], rbs[:])
                    else:
                        stg = nm_p.tile([64, QB], F32R, tag="stg")
                        nc.vector.tensor_mul(stg[:], ot[0:64, :], rbs[:])
                        nc.sync.dma_start(OTn[m][64:128, q0:q0 + QB],
                                          stg[:])

                for h in range(HL):
                    attn_head(0, h)
                    if h == 0:
                        for fn in deferred_vtrans:
                            fn(pool=ot_ps)
                for h in range(HL):
                    attn_head(1, h)
                    if h > 0:
                        # previous head's rows: deps resolved, fills PE slack
                        out_proj_qt(h - 1)
                out_proj_qt(HL - 1)
                for qt in range(QB // 128, S // 128):
                    out_proj_qt(qt, act_copy=True)

    nc.compile()
    return nc


class _Runner:
    """Persistent jitted SPMD executor (mirrors bass2jax.run_bass_via_pjrt)."""

    def __init__(self, nc):
        import jax
        import jax.core
        import concourse.mybir as mybir
        from concourse import bass2jax
        from jax.experimental.shard_map import shard_map
        from jax.sharding import Mesh, PartitionSpec

        bass2jax.install_neuronx_cc_hook()
        self.nc = nc
        in_names, out_names, out_avals, zero_outs = [], [], [], []
        part_name = (nc.partition_id_tensor.name
                     if nc.partition_id_tensor is not None else None)
        for alloc in nc.m.functions[0].allocations:
            if not isinstance(alloc, mybir.MemoryLocationSet):
                continue
            name = alloc.memorylocations[0].name
            if alloc.kind == "ExternalInput":
                if name != part_name:
                    in_names.append(name)
            elif alloc.kind == "ExternalOutput":
                out_names.append(name)
                shape = tuple(alloc.tensor_shape)
                dtype = mybir.dt.np(alloc.dtype)
                out_avals.append(jax.core.ShapedArray(shape, dtype))
                zero_outs.append(np.zeros(shape, dtype))
        self.in_names = list(in_names)
        self.out_names = out_names
        self.out_avals = out_avals
        self.zero_outs = zero_outs
        bind_in_names = tuple(
            in_names + out_names + ([part_name] if part_name else []))

        def _body(*args):
            operands = list(args)
            if part_name is not None:
                operands.append(bass2jax.partition_id_tensor())
            outs = bass2jax._bass_exec_p.bind(
                *operands,
                out_avals=tuple(out_avals),
                in_names=bind_in_names,
                out_names=tuple(out_names),
                lowering_input_output_aliases=(),
                sim_require_finite=True,
                sim_require_nnan=True,
                nc=nc,
            )
            return tuple(outs)

        self._body = _body
        devices = jax.devices()[:N_CORES]
        assert len(devices) == N_CORES, f"need {N_CORES} cores, saw {len(devices)}"
        mesh = Mesh(np.asarray(devices), ("core",))
        nin = len(in_names) + len(out_names)
        self._fn = jax.jit(
            shard_map(_body, mesh=mesh,
                      in_specs=(PartitionSpec("core"),) * nin,
                      out_specs=(PartitionSpec("core"),) * len(out_names),
                      check_rep=False),
            keep_unused=True,
        )
        self._mesh = mesh
        self._shard_map = shard_map
        self._PartitionSpec = PartitionSpec
        self._jax = jax

    def make_chain(self, k):
        """jit of k dependency-chained kernel executions (for timing)."""
        jax = self._jax
        n_in = len(self.in_names)
        n_out = len(self.out_names)

        def chain(*args):
            ins = list(args[:n_in])
            zs = list(args[n_in:])
            for _ in range(k):
                zs = list(self._body(*ins, *zs))
            return tuple(zs)

        nin = n_in + n_out
        return jax.jit(
            self._shard_map(chain, mesh=self._mesh,
                            in_specs=(self._PartitionSpec("core"),) * nin,
                            out_specs=(self._PartitionSpec("core"),) * n_out,
                            check_rep=False),
            keep_unused=True,
        )

    def prepare(self, in_maps):
        """Concatenate per-core inputs and place on device."""
        jax = self._jax
        concat = [
            np.concatenate([np.asarray(in_maps[c][n]) for c in range(N_CORES)],
                           axis=0)
            for n in self.in_names
        ]
        concat += [
            np.zeros((N_CORES * z.shape[0], *z.shape[1:]), z.dtype)
            for z in self.zero_outs
        ]
        return [jax.device_put(a) for a in concat]

    def execute(self, dev_args):
        out_arrs = self._fn(*dev_args)
        self._jax.block_until_ready(out_arrs)
        return out_arrs

    def run(self, in_maps):
        out_arrs = self.execute(self.prepare(in_maps))
        return [
            {n: np.asarray(out_arrs[i]).reshape(N_CORES, *self.out_avals[i].shape)[c]
             for i, n in enumerate(self.out_names)}
            for c in range(N_CORES)
        ]


def _head_perm():
    """Local-head row permutation: chunk m holds heads (m, m+4)."""
    order = []
    for m in range(4):
        order += list(range(64 * m, 64 * m + 64))
        order += list(range(64 * (m + 4), 64 * (m + 4) + 64))
    return np.array(order)


def prep_inputs(x, position_ids, Wq, Wk, Wv, Wo):
    x = np.asarray(x, dtype=np.float32)
    pos = np.asarray(position_ids).astype(np.float32)
    Wq = np.asarray(Wq, dtype=np.float32)
    Wk = np.asarray(Wk, dtype=np.float32)
    Wv = np.asarray(Wv, dtype=np.float32)
    Wo = np.asarray(Wo, dtype=np.float32)

    inv_freq = (1.0 / (ROPE_BASE ** (np.arange(0, D, 2, dtype=np.float32) / D))
                ).astype(np.float32)
    ang = pos[:, None] * inv_freq[None, :]          # (S, 32) fp32
    cosa = np.cos(ang).astype(np.float32)           # (S, 32)
    sina = np.sin(ang).astype(np.float32)
    ridx = np.arange(128)
    cosf = np.ascontiguousarray(cosa[:, ridx % 32].T)          # (128, S)
    sgn = np.where((ridx % 64) < 32, -1.0, 1.0).astype(np.float32)
    sinS = np.ascontiguousarray(sina[:, ridx % 32].T * sgn[:, None])

    kk = np.arange(128)[:, None]
    qq = np.arange(PB)[None, :]
    masks = np.stack([(kk <= qq - 128 * mm).astype(np.float32)
                      for mm in range(4)], axis=1)             # (128, 4, 512)
    ident = np.tile(np.eye(64, dtype=np.float32), (2, 1))      # (128, 64)
    onesc = np.ones((128, 64), dtype=np.float32)

    perm = _head_perm()
    in_maps = []
    xTb = [np.ascontiguousarray(x[b].T) for b in range(B)]     # (E, S) each
    wqTs, wkTs, wvTs, woTs = {}, {}, {}, {}
    for g in range(G):
        wq_g = Wq[512 * g:512 * (g + 1)][perm]                 # (512, E)
        wqTs[g] = np.ascontiguousarray(wq_g.T)                 # (E, 512)
        wkTs[g] = np.ascontiguousarray(Wk[128 * g:128 * (g + 1)].T)
        wvTs[g] = np.ascontiguousarray(Wv[128 * g:128 * (g + 1)].T)
        wo_g = Wo[:, 512 * g:512 * (g + 1)][:, perm]           # (E, 512)
        woTs[g] = np.ascontiguousarray(wo_g.T)                 # (512, E)
    for i in range(N_CORES):
        g, b = i // 2, i % 2
        in_maps.append({
            "xT": xTb[b], "wqT": wqTs[g], "wkT": wkTs[g],
            "wvT": wvTs[g], "woT": woTs[g],
            "cosf": cosf, "sinS": sinS, "masks": masks,
            "ident": ident, "onesc": onesc,
        })
    return in_maps


def get_runner():
    if "runner" not in _RUNTIME:
        nc = _build_nc()
        _RUNTIME["runner"] = _Runner(nc)
    return _RUNTIME["runner"]


def assemble(results):
    out = np.empty((B, S, E), dtype=np.float32)
    for b in range(B):
        acc = results[b]["out"].copy()
        for g in range(1, G):
            acc += results[2 * g + b]["out"]
        out[b] = acc
    return out


def kernel(x, position_ids, Wq, Wk, Wv, Wo):
    runner = get_runner()
    in_maps = prep_inputs(x, position_ids, Wq, Wk, Wv, Wo)
    results = runner.run(in_maps)
    return assemble(results)
